# revision 20
# baseline (speedup 1.0000x reference)
"""Multi-head attention TRN2 kernel v7 (8-core SPMD, batch x head-half).

Per core (c): batch b = c % 4, head-half g = c // 4 (8 of 16 heads),
processed as 2 passes x 2 pairs x 4 q-blocks (512 toks) = 16 attention
units of 16 key tiles each. bf16 matmul dataflow as v2/v3:
  QT/KT [2x64 hd, tok] pair-major; V token-major with ones column per
  head (softmax denominator from the attn@V matmul, row 64).

v7: globally software-pipelined single stream. Per (unit, kt) step t
the trace order is exp(t) -> QK(t+1) -> AV(t): the PE runs one score
tile ahead of ACT so exp's input is always ready. All projections
(Q/K both passes, V once for both) and the output projection are
decomposed into single-matmul micro-ops fed into per-step PE slack by
a deadline scheduler (forced when their first reader is imminent,
eager otherwise, ~2 per step; a finished PSUM chunk defers the next
chunk to the following step so its WAR on the DVE drain is off the PE
critical path). HW-measured matmul rates (this chip): K=128 N=512
~311ns (LDWEIGHTS hidden), adjacent disjoint-row K=64 ~213ns each.
Projections and the output projection therefore use full-K (K=128)
matmuls - half the moving columns of the K=64 lo/hi pair form - with a
[128,512] PSUM accumulator (1 bank, bufs=2) and a one-op DVE drain.
QK keeps the 2-head K=64 base-0/64 pair; AV keeps K=128 M=66 with the
ones column producing softmax denominators in row 64. Outputs are
written bf16 (summed f32 on host). PSUM: st 2x[128,1024] (4 banks) +
fil 2x[128,512] (2) + oAB [128,1024] (2) = 8 banks. xk is SBUF-
resident (32KB/part) so pass 1 reuses it; xq streams twice; xv once.
phases: "abc" full kernel; "ab"/"b"/"n..." are timing diagnostics.
"""
import numpy as np
import concourse.bacc as bacc
import concourse.mybir as mybir
import concourse.tile as tile
from concourse import bass_utils

BF16 = mybir.dt.bfloat16
F32 = mybir.dt.float32
AF = mybir.ActivationFunctionType

S, D = 2048, 1024
CH = 512          # token chunk for projections
NCH = S // CH     # 4
NKT = S // 128    # 16 key-token tiles
NQ = S // 512     # 4 query blocks of 512
NSTEP = 16 * NKT  # 16 units x 16 kt steps
BIG = 1 << 30


def build(reps: int = 1, phases: str = "abc"):
    nc = bacc.Bacc("TRN2", target_bir_lowering=False, debug=False, num_devices=8)
    xq_d = nc.dram_tensor("xq", [D, S], BF16, kind="ExternalInput")
    xk_d = nc.dram_tensor("xk", [D, S], BF16, kind="ExternalInput")
    xv_d = nc.dram_tensor("xv", [D, S], BF16, kind="ExternalInput")
    wq_d = nc.dram_tensor("wq", [D, 512], BF16, kind="ExternalInput")
    wk_d = nc.dram_tensor("wk", [D, 512], BF16, kind="ExternalInput")
    wv_d = nc.dram_tensor("wv", [D, 512], BF16, kind="ExternalInput")
    wo_d = nc.dram_tensor("wo", [512, D], BF16, kind="ExternalInput")
    bq_d = nc.dram_tensor("bq", [4, 128, 1], F32, kind="ExternalInput")
    bk_d = nc.dram_tensor("bk", [4, 128, 1], F32, kind="ExternalInput")
    bv_d = nc.dram_tensor("bv", [1, 512], F32, kind="ExternalInput")
    out_d = [
        nc.dram_tensor(f"out{p}", [S, D], BF16, kind="ExternalOutput")
        for p in (0, 1)
    ]

    units = [(ps, q, pp) for ps in range(2) for q in range(NQ) for pp in range(2)]

    with tile.TileContext(nc) as tc:
        with (
            tc.tile_pool(name="pers", bufs=1) as pers,
            tc.tile_pool(name="xqp", bufs=2) as xqp,
            tc.tile_pool(name="xvp", bufs=2) as xvp,
            tc.tile_pool(name="wp", bufs=2) as wp,
            tc.tile_pool(name="qkvp", bufs=2) as qkvp,
            tc.tile_pool(name="ptp", bufs=10) as ptp,
            tc.tile_pool(name="atp", bufs=2) as atp,
            tc.tile_pool(name="rbp", bufs=3) as rbp,
            tc.tile_pool(name="ocp", bufs=2) as ocp,
            tc.tile_pool(name="stp", bufs=2, space="PSUM") as stp,
            tc.tile_pool(name="op", bufs=1, space="PSUM") as op,
            tc.tile_pool(name="drp", bufs=3, space="DRAM") as drp,
        ):
            bq_sb = pers.tile([128, 4], F32, tag="bq")
            bk_sb = pers.tile([128, 4], F32, tag="bk")
            for m in range(4):
                nc.sync.dma_start(bq_sb[:, m : m + 1], bq_d[m])
                nc.sync.dma_start(bk_sb[:, m : m + 1], bk_d[m])
            bv_sb = pers.tile([128, 512], F32, tag="bv")
            nc.sync.dma_start(bv_sb[:], bv_d[:].to_broadcast((128, 512)))
            # dummy exp: pulls the one-time ACT Exp-table load into the
            # prologue so the first real softmax tile isn't delayed by it
            warm = pers.tile([128, 4], BF16, tag="warm")
            nc.scalar.activation(warm[:], bq_sb[:], AF.Exp, scale=0.0)
            # resident full xk [128, k=8 x tok] (both passes read it)
            xk_sb = pers.tile([128, 8 * S], BF16, tag="xk")

            def body():
                T = []
                for ps in range(2):
                    QT = [
                        qkvp.tile([128, S], BF16, tag=f"qt{pp}", name=f"QT{ps}{pp}")
                        for pp in range(2)
                    ]
                    KT = [
                        qkvp.tile([128, S], BF16, tag=f"kt{pp}", name=f"KT{ps}{pp}")
                        for pp in range(2)
                    ]
                    v_sb = qkvp.tile([128, NKT * 264], BF16, tag="v", name=f"v{ps}")
                    AT = [
                        atp.tile([128, S], BF16, tag=f"at{pp}", name=f"AT{ps}{pp}")
                        for pp in range(2)
                    ]
                    T.append(dict(QT=QT, KT=KT, v=v_sb, AT=AT))

                wq2, wk2, wo_t, xq_t, xv_t = {}, {}, {}, {}, {}

                def dma_wqk(ps):
                    cs = slice(ps * 256, (ps + 1) * 256)
                    wq2[ps] = wp.tile([128, 2048], BF16, tag="wq2", name=f"wq2{ps}")
                    wk2[ps] = wp.tile([128, 2048], BF16, tag="wk2", name=f"wk2{ps}")
                    if "n" in phases:
                        nc.vector.memset(wq2[ps][:, 0:8], 0.01)
                        nc.vector.memset(wk2[ps][:, 0:8], 0.01)
                        return
                    for w_sb, w_d in ((wk2[ps], wk_d), (wq2[ps], wq_d)):
                        nc.gpsimd.dma_start(
                            w_sb[:].rearrange("p (k m) -> p k m", k=8),
                            w_d[:, cs].rearrange("(k p) m -> p k m", p=128),
                        )

                def dma_wo(ps):
                    wo_sb = wp.tile([128, 2048], BF16, tag="wo", name=f"wo{ps}")
                    wo_t[ps] = wo_sb
                    if "n" in phases:
                        nc.vector.memset(wo_sb[:, 0:8], 0.01)
                        return
                    for kb in range(2):
                        rs = slice(ps * 256 + kb * 128, ps * 256 + kb * 128 + 128)
                        nc.gpsimd.dma_start(
                            wo_sb[:, kb * 1024 : (kb + 1) * 1024], wo_d[rs, :]
                        )

                def dma_xk(ch):
                    if "n" in phases:
                        nc.vector.memset(xk_sb[:, ch * 8 : ch * 8 + 8], 0.01)
                        return
                    toks = slice(ch * CH, (ch + 1) * CH)
                    nc.gpsimd.dma_start(
                        xk_sb[:].rearrange("p (k m) -> p k m", k=8)[:, :, toks],
                        xk_d[:, toks].rearrange("(k p) m -> p k m", p=128),
                    )

                def dma_xq(ps, ch):
                    toks = slice(ch * CH, (ch + 1) * CH)
                    xq_ch = xqp.tile([128, 8 * CH], BF16, tag="xq", name=f"xq{ps}{ch}")
                    xq_t[(ps, ch)] = xq_ch
                    if "n" in phases:
                        nc.vector.memset(xq_ch[:, 0:8], 0.01)
                        return
                    nc.gpsimd.dma_start(
                        xq_ch[:].rearrange("p (k m) -> p k m", k=8),
                        xq_d[:, toks].rearrange("(k p) m -> p k m", p=128),
                    )

                def dma_xv(ch):
                    toks = slice(ch * CH, (ch + 1) * CH)
                    xv_ch = xvp.tile([128, 8 * CH], BF16, tag="xv", name=f"xv{ch}")
                    xv_t[ch] = xv_ch
                    if "n" in phases:
                        nc.vector.memset(xv_ch[:, 0:8], 0.01)
                        return
                    nc.sync.dma_start(
                        xv_ch[:].rearrange("p (k m) -> p k m", k=8),
                        xv_d[:, toks].rearrange("(k p) m -> p k m", p=128),
                    )

                def memset_ones(ps):
                    for t in range(NKT):
                        vv = T[ps]["v"][:, t * 264 : (t + 1) * 264].rearrange(
                            "p (h c) -> p h c", h=4
                        )
                        nc.vector.memset(vv[:, :, 64:66], 0.0)
                        nc.vector.memset(vv[:, :, 64:65], 1.0)

                # ---- micro-op chunk builders (lists of (cost, fn)) ----
                def kq_thunks(ps, ch, pp, which):
                    # full-K (K=128) matmuls: 311ns each beats the K=64
                    # pair (426ns) since moving columns halve; LDWEIGHTS is
                    # HW-hidden. One [128,512] PSUM bank, one-op DVE fin.
                    toks = slice(ch * CH, (ch + 1) * CH)
                    m = 2 * ps + pp
                    hold = {}

                    def mk(k):
                        def f():
                            if k == 0:
                                hold["t"] = stp.tile(
                                    [128, 512], F32, tag="fil", bufs=2,
                                    name=f"f{which}{ps}{ch}{pp}",
                                )
                            pt = hold["t"]
                            w_sb = wk2[ps] if which == "k" else wq2[ps]
                            lsl = slice(
                                k * 256 + pp * 128, k * 256 + pp * 128 + 128
                            )
                            if which == "k":
                                xsl = slice(k * S + ch * CH, k * S + (ch + 1) * CH)
                                mov = xk_sb[:, xsl]
                            else:
                                xq_ch = xq_t[(ps, ch)]
                                mov = xq_ch[:, k * CH : (k + 1) * CH]
                            nc.tensor.matmul(
                                pt[:], w_sb[:, lsl], mov,
                                start=(k == 0), stop=(k == 7),
                            )

                        return (1, f)

                    def fin():
                        dest = (T[ps]["KT"] if which == "k" else T[ps]["QT"])[pp]
                        b_sb = bk_sb if which == "k" else bq_sb
                        nc.vector.tensor_scalar_add(
                            dest[:, toks], hold["t"][:], b_sb[:, m : m + 1]
                        )

                    return [mk(k) for k in range(8)] + [(0, fin)]

                def v_tt_thunks(ch, tt):
                    ti = ch * 4 + tt
                    hold = {}

                    def mk(k):
                        def f():
                            if k == 0:
                                hold["t"] = stp.tile(
                                    [128, 512], F32, tag="fil", bufs=2,
                                    name=f"av{ti}",
                                )
                            xv_ch = xv_t[ch]
                            xsl = slice(k * CH + tt * 128, k * CH + tt * 128 + 128)
                            wsl = slice(k * 512, (k + 1) * 512)
                            nc.tensor.matmul(
                                hold["t"][:], xv_ch[:, xsl], wv_full[:, wsl],
                                start=(k == 0), stop=(k == 7),
                            )

                        return (1, f)

                    def fin():
                        pt = hold["t"]
                        for ps in range(2):
                            dst = T[ps]["v"][
                                :, ti * 264 : (ti + 1) * 264
                            ].rearrange("p (h c) -> p h c", h=4)[:, :, 0:64]
                            lo = pt[:, ps * 256 : ps * 256 + 256].rearrange(
                                "p (h c) -> p h c", h=4
                            )
                            bvb = bv_sb[:, ps * 256 : (ps + 1) * 256].rearrange(
                                "p (h c) -> p h c", h=4
                            )
                            nc.vector.tensor_add(dst, lo, bvb)

                    return [mk(k) for k in range(8)] + [(0, fin)]

                def c_thunks(ps, mp):
                    # full-K matmuls (K=128 over both 64-dim halves at once):
                    # PSUM accumulates the complete sum, so the drain is one
                    # DVE copy (f32 PSUM -> bf16 SBUF) instead of copy+add x2
                    msl = slice(mp * 128, (mp + 1) * 128)
                    hold = {}

                    def mk(n, kb):
                        def f():
                            if kb == 0:
                                hold[n] = stp.tile(
                                    [128, 512], F32, tag="fil", bufs=2,
                                    name=f"c{ps}{mp}{n}",
                                )
                                if n == 0:
                                    hold["oc"] = ocp.tile(
                                        [128, 1024], BF16, tag="oc",
                                        name=f"oc{ps}{mp}",
                                    )
                            wsl = slice(
                                kb * 1024 + n * 512, kb * 1024 + n * 512 + 512
                            )
                            nc.tensor.matmul(
                                hold[n][:],
                                T[ps]["AT"][kb][:, msl], wo_t[ps][:, wsl],
                                start=(kb == 0), stop=(kb == 1),
                            )

                        return (1, f)

                    def fin(n):
                        def f():
                            nc.vector.tensor_copy(
                                hold["oc"][:, n * 512 : (n + 1) * 512], hold[n][:]
                            )
                            if n == 1:
                                nc.gpsimd.dma_start(
                                    out_d[ps][msl, :], hold["oc"][:]
                                )

                        return (0, f)

                    return [mk(0, 0), mk(0, 1), fin(0), mk(1, 0), mk(1, 1),
                            fin(1)]

                # ---- main-stream tracers ----
                st_tiles, pt_tiles, oab = {}, {}, {}

                def trace_qk(t2):
                    u2, kt2 = divmod(t2, NKT)
                    ps2, q2, pp2 = units[u2]
                    st_t = stp.tile([128, 1024], F32, tag="st", name=f"st{t2}")
                    ksl = slice(kt2 * 128, (kt2 + 1) * 128)
                    qsl = slice(q2 * 512, (q2 + 1) * 512)
                    KT, QT = T[ps2]["KT"][pp2], T[ps2]["QT"][pp2]
                    nc.tensor.matmul(
                        st_t[:, 0:512], KT[0:64, ksl], QT[0:64, qsl],
                        start=True, stop=True,
                    )
                    nc.tensor.matmul(
                        st_t[:, 512:1024], KT[64:128, ksl], QT[64:128, qsl],
                        start=True, stop=True,
                    )
                    st_tiles[t2] = st_t

                def trace_exp(t):
                    pt = ptp.tile([128, 1024], BF16, tag="pt", name=f"pt{t}")
                    nc.scalar.activation(
                        pt[:], st_tiles.pop(t)[:], AF.Exp, scale=0.125
                    )
                    pt_tiles[t] = pt

                def trace_av(t):
                    u, kt = divmod(t, NKT)
                    ps, q, pp = units[u]
                    if kt == 0:
                        oab[u] = op.tile([128, 1024], F32, tag="oAB", name=f"o{u}")
                    pt = pt_tiles.pop(t)
                    base = kt * 264 + (2 * pp) * 66
                    v_sb = T[ps]["v"]
                    nc.tensor.matmul(
                        oab[u][0:66, 0:512], v_sb[:, base : base + 66],
                        pt[:, 0:512], start=(kt == 0), stop=(kt == NKT - 1),
                    )
                    nc.tensor.matmul(
                        oab[u][0:66, 512:1024], v_sb[:, base + 66 : base + 132],
                        pt[:, 512:1024], start=(kt == 0), stop=(kt == NKT - 1),
                    )

                def drain(u):
                    ps, q, pp = units[u]
                    qsl = slice(q * 512, (q + 1) * 512)
                    oAB = oab.pop(u)
                    AT = T[ps]["AT"][pp]
                    nc.vector.tensor_copy(AT[0:64, qsl], oAB[0:64, 0:512])
                    nc.vector.tensor_copy(AT[64:128, qsl], oAB[0:64, 512:1024])
                    # denominator rows: PSUM -> SBUF -> DRAM bounce,
                    # broadcast back across partitions, reciprocal once
                    sd = rbp.tile([33, 512], F32, tag="sd", name=f"sd{u}")
                    nc.vector.tensor_copy(sd[0:1, :], oAB[64:65, 0:512])
                    nc.vector.tensor_copy(sd[32:33, :], oAB[64:65, 512:1024])
                    rc_u = drp.tile([2, 512], F32, tag="rc", name=f"rc{u}")
                    nc.sync.dma_start(rc_u[0:1, :], sd[0:1, :])
                    nc.sync.dma_start(rc_u[1:2, :], sd[32:33, :])
                    rbc = rbp.tile([128, 512], F32, tag="rbc", name=f"rbc{u}")
                    nc.sync.dma_start(
                        rbc[0:64, :], rc_u[0:1, :].to_broadcast((64, 512))
                    )
                    nc.sync.dma_start(
                        rbc[64:128, :], rc_u[1:2, :].to_broadcast((64, 512))
                    )
                    rbr = rbp.tile([128, 512], F32, tag="rbr", name=f"rbr{u}")
                    nc.vector.reciprocal(rbr[:], rbc[:])
                    nc.gpsimd.tensor_mul(AT[:, qsl], AT[:, qsl], rbr[:])

                # ---- prologue ----
                raw = []
                if "a" in phases:
                    wv_full = wp.tile(
                        [128, 4096], BF16, tag="wv", bufs=1, name="wvfull"
                    )
                    if "n" in phases:
                        nc.vector.memset(wv_full[:, 0:8], 0.01)
                    else:
                        nc.sync.dma_start(
                            wv_full[:].rearrange("p (k m) -> p k m", k=8),
                            wv_d[:, :].rearrange("(k p) m -> p k m", p=128),
                        )
                    dma_xv(0)
                    dma_xk(0)
                    dma_wqk(0)
                    dma_xq(0, 0)
                    for ch in range(1, NCH):
                        dma_xk(ch)
                    memset_ones(0)
                    memset_ones(1)
                    for _, fn in kq_thunks(0, 0, 0, "k"):
                        fn()
                    for _, fn in kq_thunks(0, 0, 0, "q"):
                        fn()
                    for _, fn in v_tt_thunks(0, 0):
                        fn()
                else:
                    # phases "b": timing diagnostic — no projections; memset
                    # QT/KT (scores=0, exp=1) and v; pure attention pipeline
                    memset_ones(0)
                    memset_ones(1)
                    for ps in range(2):
                        for pp in range(2):
                            nc.vector.memset(T[ps]["QT"][pp][:], 0.01)
                            nc.vector.memset(T[ps]["KT"][pp][:], 0.01)
                        vv = T[ps]["v"][:].rearrange(
                            "p (t h c) -> p t h c", t=NKT, h=4
                        )
                        nc.vector.memset(vv[:, :, :, 0:64], 0.01)

                # ---- filler queue: (deadline-of-last-thunk, thunks) ----
                if "a" in phases:
                    for ch in range(NCH):
                        for tt in range(4):
                            ti = ch * 4 + tt
                            if ti == 0:
                                continue
                            raw.append((ti - 1, v_tt_thunks(ch, tt)))
                        if ch >= 1:
                            raw.append(
                                (4 * ch - 6, [(0, lambda ch=ch: dma_xv(ch))])
                            )
                    for ch in range(1, NCH):
                        raw.append((4 * ch - 2, kq_thunks(0, ch, 0, "k")))
                    for ch in range(NCH):
                        raw.append((14 + 4 * ch, kq_thunks(0, ch, 1, "k")))
                    raw.append((14, kq_thunks(0, 0, 1, "q")))
                    for ch in range(1, NCH):
                        raw.append(
                            (32 * ch - 12, [(0, lambda ch=ch: dma_xq(0, ch))])
                        )
                        for pp in range(2):
                            raw.append(
                                ((2 * ch + pp) * 16 - 2, kq_thunks(0, ch, pp, "q"))
                            )
                    if "c" in phases:
                        raw.append((40, [(0, lambda: dma_wo(0))]))
                        raw.append((150, [(0, lambda: dma_wo(1))]))
                    raw.append((112, [(0, lambda: dma_wqk(1))]))
                    for ch in range(NCH):
                        raw.append(
                            (116 + 32 * ch, [(0, lambda ch=ch: dma_xq(1, ch))])
                        )
                    for ch in range(NCH):
                        for pp in range(2):
                            raw.append(
                                (126 + 16 * pp + 4 * ch, kq_thunks(1, ch, pp, "k"))
                            )
                            raw.append(
                                (126 + (2 * ch + pp) * 16, kq_thunks(1, ch, pp, "q"))
                            )
                raw.sort(key=lambda x: x[0])
                filq = []
                for dl_last, thunks in raw:
                    n = len(thunks)
                    filq.append(
                        [(dl_last - (n - 1 - i), c, f)
                         for i, (c, f) in enumerate(thunks)]
                    )

                cq = []
                pstate = dict(fi=0, cur=[])

                def pump(t, budget=2):
                    # `fresh`: a filler chunk was finished during this call.
                    # Starting another chunk now would put its first matmul
                    # (which WAR-waits on the finished chunk's DVE drain of
                    # the shared PSUM buffer) right behind the previous
                    # chunk's last matmul in the in-order PE queue -> PE
                    # head-of-line stall. Defer to the next step so main-
                    # stream ops sit in between, unless a deadline forces it.
                    fresh = False
                    while True:
                        if not pstate["cur"]:
                            fi = pstate["fi"]
                            fil_ok = fi < len(filq)
                            fil_due = fil_ok and filq[fi][0][0] <= t
                            if fresh and not fil_due:
                                return budget
                            fil_urgent = fil_ok and filq[fi][0][0] <= t + 8
                            take_fil = fil_ok and (fil_urgent or not cq)
                            if take_fil and (budget > 0 or fil_due):
                                pstate["cur"] = list(filq[fi])
                                pstate["fi"] += 1
                            elif cq and budget > 0:
                                pstate["cur"] = [
                                    (BIG, c, f) for c, f in cq.pop(0)
                                ]
                            else:
                                return budget
                        cur = pstate["cur"]
                        while cur and (budget > 0 or cur[0][0] <= t):
                            _, c, fn = cur.pop(0)
                            fn()
                            budget -= c
                        if cur:
                            return budget
                        fresh = True

                # ---- main pipelined stream ----
                trace_qk(0)
                for t in range(NSTEP):
                    u, kt = divmod(t, NKT)
                    trace_exp(t)
                    if t + 1 < NSTEP:
                        trace_qk(t + 1)
                    trace_av(t)
                    if kt == NKT - 1:
                        drain(u)
                        ps, q, pp = units[u]
                        if pp == 1 and "c" in phases:
                            for mp in range(4 * q, 4 * q + 4):
                                cq.append(c_thunks(ps, mp))
                    pump(t)
                # tail: flush all remaining fillers and c chunks
                while (
                    pstate["cur"] or pstate["fi"] < len(filq) or cq
                ):
                    pump(BIG, budget=BIG)
                if "c" not in phases:
                    # diagnostic modes: consume AT, define outputs
                    for ps in range(2):
                        oc = ocp.tile(
                            [128, 1024], BF16, tag="oc", name=f"sink{ps}"
                        )
                        nc.vector.tensor_copy(oc[:, 0:512], T[ps]["AT"][0][:, 0:512])
                        nc.vector.tensor_copy(
                            oc[:, 512:1024], T[ps]["AT"][1][:, 0:512]
                        )
                        nc.gpsimd.dma_start(out_d[ps][0:128, :], oc[:])

            if reps == 0:
                body()
            else:
                with tc.For_i(
                    0, reps, 1,
                    hint_engines=(
                        mybir.EngineType.PE,
                        mybir.EngineType.Activation,
                        mybir.EngineType.DVE,
                        mybir.EngineType.SP,
                    ),
                ):
                    body()

    nc.compile()
    return nc


def make_in_maps(query, key, value, Wq, bq, Wk, bk, Wv, bv, Wo, bo):
    """Host-side sharding: per-core input dicts (8 cores), bf16."""
    import ml_dtypes

    def bf(x):
        return np.ascontiguousarray(np.asarray(x, np.float32)).astype(
            ml_dtypes.bfloat16
        )

    qT = [bf(np.asarray(query[b]).T) for b in range(4)]
    kT = [bf(np.asarray(key[b]).T) for b in range(4)]
    vT = [bf(np.asarray(value[b]).T) for b in range(4)]
    in_maps = []
    for c in range(8):
        b, g = c % 4, c // 4
        hs = slice(g * 512, (g + 1) * 512)
        in_maps.append(
            {
                "xq": qT[b],
                "xk": kT[b],
                "xv": vT[b],
                "wq": bf(np.asarray(Wq)[hs, :].T),
                "wk": bf(np.asarray(Wk)[hs, :].T),
                "wv": bf(np.asarray(Wv)[hs, :].T),
                "wo": bf(np.asarray(Wo)[:, hs].T),
                "bq": np.ascontiguousarray(
                    np.asarray(bq, np.float32)[hs].reshape(4, 128, 1)
                ),
                "bk": np.ascontiguousarray(
                    np.asarray(bk, np.float32)[hs].reshape(4, 128, 1)
                ),
                "bv": np.ascontiguousarray(
                    np.asarray(bv, np.float32)[hs].reshape(1, 512)
                ),
            }
        )
    return in_maps


def assemble(results, bo):
    """Sum partials: out[b] = sum over half g, pass p of core partials + bo."""
    out = np.zeros((4, S, D), np.float32)
    for c in range(8):
        b = c % 4
        out[b] += np.asarray(results[c]["out0"], np.float32)
        out[b] += np.asarray(results[c]["out1"], np.float32)
    out += np.asarray(bo, np.float32)[None, None, :]
    return out


_NC_CACHE = {}


def kernel(query, key, value, Wq, bq, Wk, bk, Wv, bv, Wo, bo, *, nc=None):
    in_maps = make_in_maps(query, key, value, Wq, bq, Wk, bk, Wv, bv, Wo, bo)
    if nc is None:
        if "nc" not in _NC_CACHE:
            _NC_CACHE["nc"] = build(reps=0)
        nc = _NC_CACHE["nc"]
    res = bass_utils.run_bass_kernel_spmd(nc, in_maps, core_ids=list(range(8)))
    return assemble(res.results, bo)
